# revision 1
# baseline (speedup 1.0000x reference)
"""Trainium2 Bass kernel for nn_ModelWithSTMGNNLayer (GAT-style message passing
+ global-memory cross-attention), distributed over 8 NeuronCores.

Sharding: nodes split into 8 contiguous shards (graph parallel); edges
partitioned by destination-node owner. Within a core, dst nodes are
LPT-packed into 20 blocks of 125 real + 3 pad slots so per-block edge
counts are balanced (cap = 2048 vs 2176 unbalanced). Per layer: each core
computes its shard's h/es/ed rows with one fused matmul (stage A is
software-pipelined into the previous layer's block loop), shards are
bf16-AllGathered, edges are fetched with one dma_gather per dst block,
ed[dst] is expanded per edge by small one-hot matmuls (fp8 one-hots
streamed from DRAM), and messages are scattered with one-hot matmuls
into PSUM.
"""
import sys
for _p in ("/opt/trn_rl_repo",):
    if _p not in sys.path:
        sys.path.insert(0, _p)

import numpy as np

import concourse.bacc as bacc
import concourse.mybir as mybir
import concourse.tile as tile
from concourse.bass_utils import run_bass_kernel_spmd
from concourse.library_config import mlp

# problem constants
N, E, FEAT, C, H, D, L, T, MS, MD, NCOUT = 20000, 320000, 128, 256, 8, 32, 5, 64, 10, 128, 2
NEG = 0.2
W = 8                  # cores
NSH = N // W           # 2500 nodes per core
P = 128
NB = 20                # dst blocks per core
NREAL = NSH // NB      # 125 real nodes per block (3 pad slots)
NLOC = NB * P          # 2560 padded local nodes
# bf16 hx row: h(256) | es_hi(8) | ed_hi(8) | es_lo(8) | ed_lo(8) | pad
ROW = 384
NCH = 5                # 512-wide node chunks for x0 matmul

f32 = mybir.dt.float32
f32r = mybir.dt.float32r
bf16 = mybir.dt.bfloat16
i16 = mybir.dt.int16

# feature permutation: new col j = d*H + h  <=>  old col = h*D + d
PERM = np.array([(j % H) * D + (j // H) for j in range(C)], dtype=np.int64)


# ----------------------------------------------------------------- host prep

def _wrap_idx(v, cap):
    """dma_gather index layout: idx i at [i % 16, i // 16], tiled to 128 rows."""
    a = np.zeros((16, cap // 16), np.int16)
    i = np.arange(len(v))
    a[i % 16, i // 16] = v.astype(np.int16)
    return np.tile(a, (8, 1))


def preprocess_graph(edge_index):
    """Partition edges by dst owner; LPT-pack dst nodes into blocks of NREAL
    real slots to balance per-block edge counts. Returns cap, per-core
    (isrc, S, St) arrays and per-core slot->node permutations."""
    import ml_dtypes
    src = np.asarray(edge_index[0], dtype=np.int64)
    dst = np.asarray(edge_index[1], dtype=np.int64)
    HALF = NLOC // 2
    owner = dst // NSH
    dl = dst % NSH
    sowner = src // NSH
    sl_ = src % NSH

    deg = np.zeros((W, NSH), np.int64)
    np.add.at(deg, (owner, dl), 1)

    # LPT bin packing: per core, blocks of exactly NREAL nodes, balanced degree
    perms = np.full((W, NLOC), -1, np.int64)
    islot = np.full((W, NSH), -1, np.int64)
    for c in range(W):
        order = np.argsort(-deg[c], kind="stable")
        bsum = np.zeros(NB, np.int64)
        bcnt = np.zeros(NB, np.int64)
        for nid in order:
            open_b = np.flatnonzero(bcnt < NREAL)
            b = open_b[np.argmin(bsum[open_b])]
            s = b * P + bcnt[b]
            perms[c, s] = nid
            islot[c, nid] = s
            bsum[b] += deg[c, nid]
            bcnt[b] += 1

    slot_dst = islot[owner, dl]
    blk = slot_dst // P
    dloc = slot_dst % P
    # row id in the split-AllGather table [2, W, HALF]
    ssl = islot[sowner, sl_]
    srow = (ssl // HALF) * (W * HALF) + sowner * HALF + (ssl % HALF)

    counts = np.zeros((W, NB), np.int64)
    np.add.at(counts, (owner, blk), 1)
    cap = int(np.ceil(counts.max() / P) * P)
    ct = cap // P

    order = np.lexsort((dloc, blk, owner))
    ssrow, sdloc = srow[order], dloc[order]
    flat = (owner[order] * NB + blk[order])
    first = np.searchsorted(flat, np.arange(W * NB), side="left")

    per_core = []
    for c in range(W):
        isrc = np.zeros((NB, P, cap // 16), np.int16)
        S = np.zeros((NB, P, ct, P), ml_dtypes.float8_e4m3)
        St = np.zeros((NB, P, ct, P), ml_dtypes.float8_e4m3)
        for b in range(NB):
            g = c * NB + b
            n = int(counts[c, b])
            s0 = int(first[g])
            isrc[b] = _wrap_idx(ssrow[s0:s0 + n], cap)
            e = np.arange(n)
            p, t, d = e % P, e // P, sdloc[s0:s0 + n]
            S[b, p, t, d] = 1.0
            St[b, d, t, p] = 1.0
        per_core.append((isrc, S, St))
    return cap, per_core, perms


def preprocess_weights(inp):
    """Fold time-proj, attention vectors, memory K/Q/V into per-layer consts
    (all in the d-major permuted feature space)."""
    Wg, Wt = np.asarray(inp["Wg"]), np.asarray(inp["Wt"])
    a_src, a_dst = np.asarray(inp["a_src"]), np.asarray(inp["a_dst"])
    Wq, Wk, Wv = np.asarray(inp["Wq"]), np.asarray(inp["Wk"]), np.asarray(inp["Wv"])
    mem = np.asarray(inp["global_memory"])

    # A_src[c=h*D+d, h2] = a[h2, d] * (h == h2)
    def a_mat(a_l):
        A = np.zeros((C, H), np.float32)
        for h in range(H):
            A[h * D:(h + 1) * D, h] = a_l[h]
        return A

    rhs_all, wqk_all, v_all = [], [], []
    for l in range(L):
        R_es = Wg[l] @ a_mat(a_src[l])            # [C, H]
        R_ed = Wg[l] @ a_mat(a_dst[l])
        Wg_p = Wg[l][np.ix_(PERM, PERM)]          # rows+cols permuted
        Rx = np.concatenate([Wg_p, R_es[PERM], R_ed[PERM]], axis=1)   # [256, 272]
        Rte = np.concatenate([Wt[l] @ Wg[l][:, PERM], Wt[l] @ R_es, Wt[l] @ R_ed],
                             axis=1)              # [64, 272]
        rhs = np.zeros((3, P, C + 2 * H), np.float32)
        rhs[0] = Rx[:P]
        rhs[1] = Rx[P:]
        rhs[2, :T] = Rte
        rhs_all.append(rhs)

        k = mem @ Wk[l]                            # [MS, C]
        wqk = (Wq[l] @ k.T) / np.sqrt(C)           # [C, MS]
        wqk_all.append(wqk[PERM].astype(np.float32))
        v = mem @ Wv[l]                            # [MS, C]
        v_all.append(v[:, PERM].astype(np.float32))
    return (np.stack(rhs_all).astype(np.float32),          # [L, 3, 128, 272]
            np.stack(wqk_all),                             # [L, 256, 10]
            np.stack(v_all))                               # [L, 10, 256]


# ------------------------------------------------------------------ builder

def build_nc(ct, l_run=L, f8_onehot=True):
    """ct = cap // 128 edge tiles per block."""
    cap = ct * P
    f8 = mybir.dt.float8e4 if f8_onehot else mybir.dt.bfloat16
    nc = bacc.Bacc("TRN2", num_devices=W)

    xiT = nc.dram_tensor("xiT", [FEAT, NLOC], f32r, kind="ExternalInput")
    teT = nc.dram_tensor("teT", [T, NLOC], f32r, kind="ExternalInput")
    wi = nc.dram_tensor("wi", [FEAT, C], f32r, kind="ExternalInput")
    rhs_d = nc.dram_tensor("rhs", [l_run, 3, P, C + 2 * H], f32r, kind="ExternalInput")
    wqk_d = nc.dram_tensor("wqk", [l_run, C, MS], f32r, kind="ExternalInput")
    v_d = nc.dram_tensor("v", [l_run, MS, C], f32r, kind="ExternalInput")
    ident_d = nc.dram_tensor("ident", [P, P], f32r, kind="ExternalInput")
    isrc_d = nc.dram_tensor("isrc", [NB, P, cap // 16], i16, kind="ExternalInput")
    S_d = nc.dram_tensor("S", [NB, P, ct, P], f8, kind="ExternalInput")
    St_d = nc.dram_tensor("St", [NB, P, ct, P], f8, kind="ExternalInput")

    pooled_d = nc.dram_tensor("pooled", [P, 2], f32, kind="ExternalOutput")

    AluOp, ActF, AxL = mybir.AluOpType, mybir.ActivationFunctionType, mybir.AxisListType

    with tile.TileContext(nc) as tc:
        with (
            tc.tile_pool(name="const", bufs=1) as cst,
            tc.tile_pool(name="xt", bufs=2) as xtp,
            tc.tile_pool(name="stage_a", bufs=2) as sta,
            tc.tile_pool(name="hx", bufs=2) as hxp,
            tc.tile_pool(name="gather", bufs=3) as gat,
            tc.tile_pool(name="onehot", bufs=3) as ohp,
            tc.tile_pool(name="edge", bufs=3) as edg,
            tc.tile_pool(name="node", bufs=3) as nod,
            tc.tile_pool(name="small", bufs=3) as sml,
            tc.tile_pool(name="psA", bufs=2, space="PSUM") as psA,
            tc.tile_pool(name="psB", bufs=2, space="PSUM") as psB,
            tc.tile_pool(name="psC", bufs=1, space="PSUM") as psC,
            tc.tile_pool(name="dram", bufs=1, space="DRAM") as drm,
        ):
            nc.gpsimd.load_library(mlp)

            # ---------- constants to SBUF
            ident = cst.tile([P, P], f32r)
            nc.sync.dma_start(ident[:], ident_d[:])
            wqk_sb = cst.tile([P, l_run * 2, MS], f32r)
            nc.sync.dma_start(wqk_sb[:], wqk_d[:].rearrange("l (k p) m -> p (l k) m", p=P))
            v_sb = cst.tile([MS, l_run, C], f32r)
            nc.sync.dma_start(v_sb[:], v_d[:].rearrange("l m f -> m l f"))
            teT_sb = cst.tile([T, NLOC], f32r)
            nc.sync.dma_start(teT_sb[:], teT[:])
            wi_sb = cst.tile([P, 2, P], f32r)
            nc.sync.dma_start(wi_sb[:], wi[:].rearrange("f (k p) -> f k p", k=2))
            isrc_sb = cst.tile([P, NB, cap // 16], i16)
            nc.sync.dma_start(isrc_sb[:], isrc_d[:].rearrange("b p f -> p b f"))

            # ---------- DRAM internal buffers
            ag_in = drm.tile([NLOC, ROW], bf16)

            def new_hx():
                t_ = hxp.tile([P, NB, ROW], bf16, tag="hx")
                nc.vector.memset(t_[:, :, C + 4 * H:ROW], 0.0)
                return t_

            def stage_a_tile(lx, t, xT_src, rhs_sb, hx_sb):
                h_ps = psA.tile([P, 512], f32, space="PSUM", tag="A",
                                name="h_ps")[:, 0:C + 2 * H]
                sl = slice(t * P, (t + 1) * P)
                nc.tensor.matmul(h_ps[:], xT_src[:, 0, sl], rhs_sb[:, 0, :],
                                 start=True, stop=False)
                nc.tensor.matmul(h_ps[:], xT_src[:, 1, sl], rhs_sb[:, 1, :],
                                 start=False, stop=False)
                nc.tensor.matmul(h_ps[:], teT_sb[:, sl], rhs_sb[:T, 2, :],
                                 start=False, stop=True)
                # h payload + hi pair (es|ed) on scalar, lo pair on vector
                nc.scalar.activation(hx_sb[:, t, 0:C], h_ps[:, 0:C], ActF.Copy)
                nc.scalar.activation(hx_sb[:, t, C:C + 2 * H],
                                     h_ps[:, C:C + 2 * H], ActF.Copy)
                nc.vector.tensor_tensor(out=hx_sb[:, t, C + 2 * H:C + 4 * H],
                                        in0=h_ps[:, C:C + 2 * H],
                                        in1=hx_sb[:, t, C:C + 2 * H],
                                        op=AluOp.subtract)

            HALF = NLOC // 2
            NBH = NB // 2

            def new_hx_full(lx):
                return drm.tile([2, W, HALF, ROW], bf16,
                                tag=f"hxf{lx}", name=f"hx_full{lx}")

            def ag_half(lx, hx_sb, hx_full, half):
                lo, hi = (0, NBH) if half == 0 else (NBH, NB)
                nc.sync.dma_start(
                    ag_in[half * HALF:(half + 1) * HALF]
                        .rearrange("(t p) f -> p t f", p=P),
                    hx_sb[:, lo:hi, :])
                nc.gpsimd.collective_compute(
                    "AllGather", AluOp.bypass,
                    replica_groups=[list(range(W))],
                    ins=[ag_in[half * HALF:(half + 1) * HALF]],
                    outs=[hx_full[half].rearrange("w r f -> (w r) f")])

            # ---------- clear gather buffers once (stale pad lanes must be finite)
            for _ in range(3):
                g0 = gat.tile([P, ct, ROW], bf16, tag="G")
                nc.vector.memset(g0[:], 0.0)

            # ---------- x0T = relu(Wi'.T @ xiT), interleaved with stage A l=0
            rhs_sb = sta.tile([P, 3, C + 2 * H], f32r, tag="rhs")
            nc.sync.dma_start(rhs_sb[:], rhs_d[0].rearrange("k p f -> p k f"))
            hx_sb = new_hx()
            xT = xtp.tile([P, 2, NLOC], f32r, tag="xT")
            for j in range(NCH):
                xi_sb = sml.tile([FEAT, 512], f32r, tag="xi")
                nc.sync.dma_start(xi_sb[:], xiT[:, j * 512:(j + 1) * 512])
                for k in range(2):
                    x0_ps = psA.tile([P, 512], f32, space="PSUM", tag="A")
                    nc.tensor.matmul(x0_ps[:], wi_sb[:, k, :], xi_sb[:],
                                     start=True, stop=True)
                    nc.scalar.activation(xT[:, k, j * 512:(j + 1) * 512], x0_ps[:],
                                         ActF.Relu)
                for t in range(4 * j, 4 * j + 4):
                    stage_a_tile(0, t, xT, rhs_sb, hx_sb)
                if j == 2:
                    hx_full = new_hx_full(0)
                    ag_half(0, hx_sb, hx_full, 0)
            ag_half(0, hx_sb, hx_full, 1)

            # ---------------- layers
            for l in range(l_run):
                last = (l + 1 == l_run)
                if not last:
                    rhs_next = sta.tile([P, 3, C + 2 * H], f32r, tag="rhs")
                    nc.sync.dma_start(rhs_next[:],
                                      rhs_d[l + 1].rearrange("k p f -> p k f"))
                    hx_next = new_hx()
                xT_new = xtp.tile([P, 2, NLOC], f32r, tag="xT")

                def front(b):
                    """Gather + logits + weighted-feature build for block b."""
                    S_sb = ohp.tile([P, ct, P], f8, tag="S")
                    nc.sync.dma_start(S_sb[:], S_d[b])
                    St_sb = ohp.tile([P, ct, P], f8, tag="St")
                    nc.sync.dma_start(St_sb[:], St_d[b])

                    G = gat.tile([P, ct, ROW], bf16, tag="G")
                    nc.gpsimd.dma_gather(
                        G[:], hx_full[:].rearrange("a w r f -> (a w r) f"),
                        isrc_sb[:, b, :], cap, cap, ROW, single_packet=False)

                    # ed-expansion: ED[e, 0:8]=ed_hi, [8:16]=junk, [16:24]=ed_lo
                    EDt = psA.tile([P, 512], f32, space="PSUM", tag="A",
                                   name="EDt")[:, 0:ct * 24]
                    EDv = EDt.rearrange("p (t k) -> p t k", k=24)
                    for t in range(ct):
                        nc.tensor.matmul(EDv[:, t, :], St_sb[:, t, :],
                                         hx_sb[:, b, C + H:C + 4 * H],
                                         start=True, stop=True)

                    # logits = es_hi + es_lo + ed_hi + ed_lo; leaky; exp
                    LG = edg.tile([P, ct, H], f32, tag="LG")
                    nc.vector.tensor_tensor(out=LG[:], in0=G[:, :, C:C + H],
                                            in1=G[:, :, C + 2 * H:C + 3 * H],
                                            op=AluOp.add)
                    nc.vector.tensor_tensor(out=LG[:], in0=LG[:], in1=EDv[:, :, 0:8],
                                            op=AluOp.add)
                    nc.vector.tensor_tensor(out=LG[:], in0=LG[:], in1=EDv[:, :, 16:24],
                                            op=AluOp.add)
                    LR = edg.tile([P, ct, H], f32, tag="LR")
                    nc.vector.scalar_tensor_tensor(
                        out=LR[:], in0=LG[:], scalar=NEG, in1=LG[:],
                        op0=AluOp.mult, op1=AluOp.max)
                    # WF = [ ex * h  |  ex ]
                    WF = edg.tile([P, ct, C + H], bf16, tag="WF")
                    nc.scalar.activation(WF[:, :, C:C + H], LR[:], ActF.Exp)
                    nc.vector.tensor_tensor(
                        out=WF[:, :, 0:C].rearrange("p n (d h) -> p n d h", h=H),
                        in0=G[:, :, 0:C].rearrange("p n (d h) -> p n d h", h=H),
                        in1=WF[:, :, C:C + H].rearrange("p n (o h) -> p n o h", o=1)
                              .broadcast_to([P, ct, D, H]),
                        op=AluOp.mult)
                    return S_sb, WF

                def back(b, S_sb, WF):
                    """Scatter + normalize + node stage (+ next-layer stage A)."""
                    msg_ps = psB.tile([P, C + H], f32, space="PSUM", tag="msg")
                    for t in range(ct):
                        nc.tensor.matmul(msg_ps[:], S_sb[:, t, :], WF[:, t, :],
                                         start=(t == 0), stop=(t == ct - 1))

                    # normalize + relu -> node_new (fused: relu(msg)*recip)
                    recip = sml.tile([P, H], f32, tag="recip")
                    nc.vector.tensor_scalar(out=recip[:], in0=msg_ps[:, C:C + H],
                                            scalar1=1e-12, scalar2=None, op0=AluOp.add)
                    nc.vector.reciprocal(recip[:], recip[:])
                    node_new = nod.tile([P, C], f32, tag="nn")
                    nc.vector.scalar_tensor_tensor(
                        out=node_new[:].rearrange("p (d h) -> p d h", h=H),
                        in0=msg_ps[:, 0:C].rearrange("p (d h) -> p d h", h=H),
                        scalar=0.0,
                        in1=recip[:].rearrange("p (o h) -> p o h", o=1)
                              .broadcast_to([P, D, H]),
                        op0=AluOp.max, op1=AluOp.mult)

                    # nnT via its own transpose group (for the Wqk scores)
                    nt_ps = psB.tile([P, 2, P], f32, space="PSUM", tag="scat",
                                     name="nt_ps", bufs=2)
                    nnT = [sml.tile([P, P], f32r, tag=f"nnT{k}", name=f"nnT{k}")
                           for k in range(2)]
                    for k in range(2):
                        nc.tensor.matmul(nt_ps[:, k, :], node_new[:, k * P:(k + 1) * P],
                                         ident[:].bitcast(f32), is_transpose=True,
                                         start=True, stop=True)
                        nc.vector.tensor_copy(nnT[k][:], nt_ps[:, k, :])
                    # scores = node' @ Wqk'  -> [128 nodes, MS]
                    sc_ps = psB.tile([P, MS], f32, space="PSUM", tag="scat", bufs=2)
                    nc.tensor.matmul(sc_ps[:], nnT[0][:], wqk_sb[:, l * 2 + 0, :],
                                     start=True, stop=False)
                    nc.tensor.matmul(sc_ps[:], nnT[1][:], wqk_sb[:, l * 2 + 1, :],
                                     start=False, stop=True)
                    # softmax over MS (no max-sub; scores bounded)
                    attn = sml.tile([P, MS], f32, tag="attn")
                    dnm = sml.tile([P, 1], f32, tag="dnm")
                    nc.scalar.activation(attn[:], sc_ps[:], ActF.Exp, accum_out=dnm[:])
                    nc.vector.reciprocal(dnm[:], dnm[:])
                    nc.vector.tensor_scalar(out=attn[:], in0=attn[:], scalar1=dnm[:, 0:1],
                                            scalar2=None, op0=AluOp.mult)
                    # attnT [MS, 128]
                    at_ps = psB.tile([MS, P], f32, space="PSUM", tag="scat",
                                     name="at_ps", bufs=2)
                    nc.tensor.matmul(at_ps[:], attn[:], ident[:].bitcast(f32),
                                     is_transpose=True, start=True, stop=True)
                    attnT = sml.tile([MS, P], f32r, tag="attnT")
                    nc.vector.tensor_copy(attnT[:], at_ps[:])
                    # accumulate attnvT + xT + nnT into outT psum; relu -> xT_new
                    o_all = psC.tile([P, 2, 512], f32, space="PSUM", tag="ops")
                    o_ps = [o_all[:, k, 0:P] for k in range(2)]
                    sl = slice(b * P, (b + 1) * P)
                    for k in range(2):
                        nc.tensor.matmul(o_ps[k], v_sb[:, l, k * P:(k + 1) * P],
                                         attnT[:], start=True, stop=False)
                        nc.tensor.matmul(o_ps[k], ident[:], xT[:, k, sl],
                                         start=False, stop=False)
                        nc.tensor.matmul(o_ps[k], node_new[:, k * P:(k + 1) * P],
                                         ident[:].bitcast(f32), is_transpose=True,
                                         start=False, stop=True)
                        nc.scalar.activation(xT_new[:, k, sl], o_ps[k], ActF.Relu)

                    # software-pipelined stage A for the next layer
                    if not last:
                        stage_a_tile(l + 1, b, xT_new, rhs_next, hx_next)
                        if b == NBH - 1:
                            nonlocal_state["hxf_next"] = new_hx_full(l + 1)
                            ag_half(l + 1, hx_next, nonlocal_state["hxf_next"], 0)

                # one-block-deep software pipeline: front(b) || back(b-1)
                nonlocal_state = {}
                carry = front(0)
                for b in range(1, NB):
                    nxt = front(b)
                    back(b - 1, *carry)
                    carry = nxt
                back(NB - 1, *carry)
                if not last:
                    ag_half(l + 1, hx_next, nonlocal_state["hxf_next"], 1)
                    hx_full = nonlocal_state["hxf_next"]
                    hx_sb = hx_next
                    rhs_sb = rhs_next
                xT = xT_new

            # ---------- pooled partial (exclude per-block pad slots)
            pr1 = sml.tile([P, 2, NB], f32, tag="pr1")
            nc.vector.tensor_reduce(
                out=pr1[:],
                in_=xT[:].rearrange("p k (b q) -> p k b q", q=P)[:, :, :, 0:NREAL],
                axis=AxL.X, op=AluOp.add)
            pooled = sml.tile([P, 2], f32, tag="pooled")
            nc.vector.tensor_reduce(out=pooled[:], in_=pr1[:],
                                    axis=AxL.X, op=AluOp.add)
            nc.sync.dma_start(pooled_d[:], pooled[:])

    nc.compile()
    return nc


# ------------------------------------------------------------------ driver

_CACHED = {}
LAST_EXEC_NS = None


def kernel(**inputs):
    x0 = np.asarray(inputs["x_initial_nodes"], np.float32)
    te = np.asarray(inputs["time_embedding"], np.float32)
    wi = np.asarray(inputs["Wi"], np.float32)[:, PERM].copy()
    bi = np.asarray(inputs["bi"], np.float32)
    assert np.abs(bi).max() == 0.0, "kernel assumes bi == 0"
    rhs, wqk, v = preprocess_weights(inputs)
    cap, per_core, perms = preprocess_graph(np.asarray(inputs["edge_index"]))
    ct = cap // P

    ident = np.eye(P, dtype=np.float32)

    import os
    f8_onehot = os.environ.get("KERNEL_F8", "1") == "1"
    if not f8_onehot:
        import ml_dtypes
        per_core = [(isrc, S.astype(ml_dtypes.bfloat16), St.astype(ml_dtypes.bfloat16))
                    for isrc, S, St in per_core]

    in_maps = []
    for c in range(W):
        pm = perms[c]
        sel = np.where(pm >= 0, pm, 0)
        mask = (pm >= 0).astype(np.float32)
        xiT = (x0[c * NSH:(c + 1) * NSH].T[:, sel] * mask).astype(np.float32)
        teT = (te[c * NSH:(c + 1) * NSH].T[:, sel] * mask).astype(np.float32)
        isrc, S, St = per_core[c]
        in_maps.append({
            "xiT": xiT, "teT": teT, "wi": wi,
            "rhs": rhs, "wqk": wqk, "v": v,
            "ident": ident,
            "isrc": isrc, "S": S, "St": St,
        })

    key = (ct, f8_onehot)
    if key not in _CACHED:
        _CACHED[key] = build_nc(ct, f8_onehot=f8_onehot)
    nc = _CACHED[key]
    trace = os.environ.get("KERNEL_TRACE", "0") == "1"
    tdir = os.environ.get("KERNEL_TRACE_DIR") or None
    res = run_bass_kernel_spmd(nc, in_maps, core_ids=list(range(W)), trace=trace,
                               tmpdir=tdir)
    global LAST_EXEC_NS
    LAST_EXEC_NS = res.exec_time_ns

    # feature j_new = k*128 + p
    pooled_new = np.zeros(C, np.float64)
    for c in range(W):
        po = res.results[c]["pooled"].astype(np.float64)
        pooled_new[0:P] += po[:, 0]
        pooled_new[P:C] += po[:, 1]
    pooled_old = np.empty(C, np.float64)
    pooled_old[PERM] = pooled_new
    pooled_old /= N

    mem = np.asarray(inputs["global_memory"], np.float32)
    mem_pooled = mem.mean(axis=0)
    final = np.concatenate([pooled_old.astype(np.float32), mem_pooled])
    out = final @ np.asarray(inputs["Wc"], np.float32) + np.asarray(inputs["bc"], np.float32)
    return out.astype(np.float32)


if __name__ == "__main__":
    import reference
    inp = {k: np.asarray(v) for k, v in reference.setup_inputs().items()}
    got = kernel(**inp)
    exp = np.asarray(reference.reference(**reference.setup_inputs()))
    err = np.abs(got - exp).max() / (np.abs(exp).max() + 1e-12)
    print("kernel:", got, "\nref:   ", exp, "\nrel err:", err)



# revision 44
# speedup vs baseline: 1.1597x; 1.1597x over previous
"""Trainium2 Bass kernel for nn_ModelWithSTMGNNLayer (GAT-style message passing
+ global-memory cross-attention), distributed over 8 NeuronCores.

Sharding: nodes split into 8 contiguous shards (graph parallel); edges
partitioned by destination-node owner. Within a core, dst nodes are
LPT-packed into 20 blocks of 125 real + 3 pad slots so per-block edge
counts are balanced (cap = 2048 vs 2176 unbalanced). Per layer: each core
computes its shard's h/es/ed rows with one fused matmul (stage A is
software-pipelined into the previous layer's block loop), shards are
bf16-AllGathered, edges are fetched with one dma_gather per dst block,
ed[dst] is expanded per edge by small one-hot matmuls (fp8 one-hots
streamed from DRAM), and messages are scattered with one-hot matmuls
into PSUM.
"""
import sys
for _p in ("/opt/trn_rl_repo",):
    if _p not in sys.path:
        sys.path.insert(0, _p)

import numpy as np

import concourse.bacc as bacc
import concourse.mybir as mybir
import concourse.tile as tile
from concourse.bass_utils import run_bass_kernel_spmd
from concourse.library_config import mlp

# problem constants
N, E, FEAT, C, H, D, L, T, MS, MD, NCOUT = 20000, 320000, 128, 256, 8, 32, 5, 64, 10, 128, 2
NEG = 0.2
W = 8                  # cores
NSH = N // W           # 2500 nodes per core
P = 128
NB = 20                # dst blocks per core
NREAL = NSH // NB      # 125 real nodes per block (3 pad slots)
NLOC = NB * P          # 2560 padded local nodes
# hx row (512 B = min 256B-multiple): h fp8 (256 B = bf16 cols 0:128) |
# es_hi|ed_hi bf16 (cols 128:144) | es_lo|ed_lo bf16 (cols 144:160) | pad
ROW = 256              # bf16 cols per row (512 B)
EH = 128               # bf16 col where es_hi starts
EL = 144               # bf16 col where es_lo starts
NCH = 5                # 512-wide node chunks for x0 matmul
GBUFS = 6              # gather buffers
PRE = 3                # gather descriptor-gen lookahead (prepare_only)
# AllGather chunk boundaries in dst-block units; the small final chunk keeps
# the next layer's first gather off the critical path.
CHB = ((0, 8), (8, 14), (14, 18), (18, 20))
CH_ROWS = tuple((e - s) * P for s, e in CHB)            # rows per chunk per core
CH_BASE = tuple(int(np.sum([W * r for r in CH_ROWS[:i]])) for i in range(len(CHB)))
TOT_ROWS = W * NLOC

f32 = mybir.dt.float32
f32r = mybir.dt.float32r
bf16 = mybir.dt.bfloat16
i16 = mybir.dt.int16
f8e4 = mybir.dt.float8e4

# feature permutation: new col j = d*H + h  <=>  old col = h*D + d
PERM = np.array([(j % H) * D + (j // H) for j in range(C)], dtype=np.int64)


# ----------------------------------------------------------------- host prep

def _wrap_idx(v, cap):
    """dma_gather index layout: idx i at [i % 16, i // 16], tiled to 128 rows.
    Trailing slots are -1: the gather ucode trims trailing negatives, skipping
    descriptor generation + DMA for pad lanes (their G lanes keep stale data;
    S one-hot columns are zero there so they never scatter)."""
    a = np.full((16, cap // 16), -1, np.int16)
    i = np.arange(len(v))
    a[i % 16, i // 16] = v.astype(np.int16)
    return np.tile(a, (8, 1))


def preprocess_graph(edge_index):
    """Partition edges by dst owner; LPT-pack dst nodes into blocks of NREAL
    real slots to balance per-block edge counts. Returns cap, per-core
    (isrc, S, St) arrays and per-core slot->node permutations."""
    import ml_dtypes
    src = np.asarray(edge_index[0], dtype=np.int64)
    dst = np.asarray(edge_index[1], dtype=np.int64)
    owner = dst // NSH
    dl = dst % NSH
    sowner = src // NSH
    sl_ = src % NSH

    deg = np.zeros((W, NSH), np.int64)
    np.add.at(deg, (owner, dl), 1)

    # LPT bin packing: per core, blocks of exactly NREAL nodes, balanced degree
    perms = np.full((W, NLOC), -1, np.int64)
    islot = np.full((W, NSH), -1, np.int64)
    for c in range(W):
        order = np.argsort(-deg[c], kind="stable")
        bsum = np.zeros(NB, np.int64)
        bcnt = np.zeros(NB, np.int64)
        for nid in order:
            open_b = np.flatnonzero(bcnt < NREAL)
            b = open_b[np.argmin(bsum[open_b])]
            s = b * P + bcnt[b]
            perms[c, s] = nid
            islot[c, nid] = s
            bsum[b] += deg[c, nid]
            bcnt[b] += 1

    slot_dst = islot[owner, dl]
    blk = slot_dst // P
    dloc = slot_dst % P
    # row id in the chunked-AllGather table: chunk a holds blocks
    # CHB[a], laid out [W, CH_ROWS[a]] at CH_BASE[a]
    ssl = islot[sowner, sl_]
    sblk = ssl // P
    chunk = np.zeros_like(ssl)
    for a, (s_, e_) in enumerate(CHB):
        chunk[(sblk >= s_) & (sblk < e_)] = a
    ch_base = np.asarray(CH_BASE)[chunk]
    ch_rows = np.asarray(CH_ROWS)[chunk]
    ch_start = np.asarray([s_ * P for s_, _ in CHB])[chunk]
    srow = ch_base + sowner * ch_rows + (ssl - ch_start)

    counts = np.zeros((W, NB), np.int64)
    np.add.at(counts, (owner, blk), 1)
    cap = int(np.ceil(counts.max() / P) * P)
    ct = cap // P

    order = np.lexsort((dloc, blk, owner))
    ssrow, sdloc = srow[order], dloc[order]
    flat = (owner[order] * NB + blk[order])
    first = np.searchsorted(flat, np.arange(W * NB), side="left")

    per_core = []
    for c in range(W):
        isrc = np.zeros((NB, P, cap // 16), np.int16)
        S = np.zeros((NB, P, ct, P), ml_dtypes.float8_e4m3)
        St = np.zeros((NB, P, ct, P), ml_dtypes.float8_e4m3)
        for b in range(NB):
            g = c * NB + b
            n = int(counts[c, b])
            s0 = int(first[g])
            isrc[b] = _wrap_idx(ssrow[s0:s0 + n], cap)
            e = np.arange(n)
            p, t, d = e % P, e // P, sdloc[s0:s0 + n]
            S[b, p, t, d] = 1.0
            St[b, d, t, p] = 1.0
        per_core.append((isrc, S, St))
    return cap, per_core, perms


def preprocess_weights(inp):
    """Fold time-proj, attention vectors, memory K/Q/V into per-layer consts
    (all in the d-major permuted feature space)."""
    Wg, Wt = np.asarray(inp["Wg"]), np.asarray(inp["Wt"])
    a_src, a_dst = np.asarray(inp["a_src"]), np.asarray(inp["a_dst"])
    Wq, Wk, Wv = np.asarray(inp["Wq"]), np.asarray(inp["Wk"]), np.asarray(inp["Wv"])
    mem = np.asarray(inp["global_memory"])

    # A_src[c=h*D+d, h2] = a[h2, d] * (h == h2)
    def a_mat(a_l):
        A = np.zeros((C, H), np.float32)
        for h in range(H):
            A[h * D:(h + 1) * D, h] = a_l[h]
        return A

    rhs_all, wqk_all, v_all = [], [], []
    for l in range(L):
        R_es = Wg[l] @ a_mat(a_src[l])            # [C, H]
        R_ed = Wg[l] @ a_mat(a_dst[l])
        Wg_p = Wg[l][np.ix_(PERM, PERM)]          # rows+cols permuted
        Rx = np.concatenate([Wg_p, R_es[PERM], R_ed[PERM]], axis=1)   # [256, 272]
        Rte = np.concatenate([Wt[l] @ Wg[l][:, PERM], Wt[l] @ R_es, Wt[l] @ R_ed],
                             axis=1)              # [64, 272]
        rhs = np.zeros((3, P, C + 2 * H), np.float32)
        rhs[0] = Rx[:P]
        rhs[1] = Rx[P:]
        rhs[2, :T] = Rte
        rhs_all.append(rhs)

        k = mem @ Wk[l]                            # [MS, C]
        wqk = (Wq[l] @ k.T) / np.sqrt(C)           # [C, MS]
        wqk_all.append(wqk[PERM].astype(np.float32))
        v = mem @ Wv[l]                            # [MS, C]
        v_all.append(v[:, PERM].astype(np.float32))
    return (np.stack(rhs_all).astype(np.float32),          # [L, 3, 128, 272]
            np.stack(wqk_all),                             # [L, 256, 10]
            np.stack(v_all))                               # [L, 10, 256]


# ------------------------------------------------------------------ builder

def build_nc(ct, l_run=L, f8_onehot=True, use_prep=True, debug=False):
    """ct = cap // 128 edge tiles per block."""
    cap = ct * P
    f8 = mybir.dt.float8e4 if f8_onehot else mybir.dt.bfloat16
    nc = bacc.Bacc("TRN2", num_devices=W, num_swdge_queues=2,
                   dynamic_dma_scratch_size=32768)
    if debug:
        dbg_hx = nc.dram_tensor("dbg_hx", [P, NB, ROW], bf16, kind="ExternalOutput")
        dbg_g = nc.dram_tensor("dbg_g", [P, ct, ROW], bf16, kind="ExternalOutput")
        dbg_xt = nc.dram_tensor("dbg_xt", [P, 2, NLOC], f32, kind="ExternalOutput")

    xiT = nc.dram_tensor("xiT", [FEAT, NLOC], f32r, kind="ExternalInput")
    teT = nc.dram_tensor("teT", [T, NLOC], f32r, kind="ExternalInput")
    wi = nc.dram_tensor("wi", [FEAT, C], f32r, kind="ExternalInput")
    rhs_d = nc.dram_tensor("rhs", [l_run, 3, P, C + 2 * H], f32r, kind="ExternalInput")
    wqk_d = nc.dram_tensor("wqk", [l_run, C, MS], f32r, kind="ExternalInput")
    v_d = nc.dram_tensor("v", [l_run, MS, C], f32r, kind="ExternalInput")
    ident_d = nc.dram_tensor("ident", [P, P], f32r, kind="ExternalInput")
    isrc_d = nc.dram_tensor("isrc", [NB, P, cap // 16], i16, kind="ExternalInput")
    S_d = nc.dram_tensor("S", [NB, P, ct, P], f8, kind="ExternalInput")
    St_d = nc.dram_tensor("St", [NB, P, ct, P], f8, kind="ExternalInput")

    pooled_d = nc.dram_tensor("pooled", [P, 2], f32, kind="ExternalOutput")

    AluOp, ActF, AxL = mybir.AluOpType, mybir.ActivationFunctionType, mybir.AxisListType

    with tile.TileContext(nc) as tc:
        with (
            tc.tile_pool(name="const", bufs=1) as cst,
            tc.tile_pool(name="xt", bufs=2) as xtp,
            tc.tile_pool(name="stage_a", bufs=2) as sta,
            tc.tile_pool(name="hx", bufs=2) as hxp,
            tc.tile_pool(name="gather", bufs=GBUFS) as gat,
            tc.tile_pool(name="onehot", bufs=3) as ohp,
            tc.tile_pool(name="edge", bufs=3) as edg,
            tc.tile_pool(name="node", bufs=3) as nod,
            tc.tile_pool(name="small", bufs=3) as sml,
            tc.tile_pool(name="psA", bufs=2, space="PSUM") as psA,
            tc.tile_pool(name="psB", bufs=2, space="PSUM") as psB,
            tc.tile_pool(name="psC", bufs=1, space="PSUM") as psC,
            tc.tile_pool(name="dram", bufs=1, space="DRAM") as drm,
        ):
            nc.gpsimd.load_library(mlp)
            gsem = nc.alloc_semaphore("gather_dma")
            nc.gpsimd.sem_clear(gsem)

            # ---------- constants to SBUF
            ident = cst.tile([P, P], f32r)
            nc.sync.dma_start(ident[:], ident_d[:])
            wqk_sb = cst.tile([P, l_run * 2, MS], f32r)
            nc.sync.dma_start(wqk_sb[:], wqk_d[:].rearrange("l (k p) m -> p (l k) m", p=P))
            v_sb = cst.tile([MS, l_run, C], f32r)
            nc.sync.dma_start(v_sb[:], v_d[:].rearrange("l m f -> m l f"))
            teT_sb = cst.tile([T, NLOC], f32r)
            nc.sync.dma_start(teT_sb[:], teT[:])
            wi_sb = cst.tile([P, 2, P], f32r)
            nc.sync.dma_start(wi_sb[:], wi[:].rearrange("f (k p) -> f k p", k=2))
            isrc_sb = cst.tile([P, NB, cap // 16], i16)
            nc.sync.dma_start(isrc_sb[:], isrc_d[:].rearrange("b p f -> p b f"))

            # ---------- DRAM internal buffers
            ag_in = drm.tile([NLOC, ROW], bf16)

            def new_hx():
                # zero the pad cols: they are DMA'd through the AllGather and
                # the gather; uninitialized SBUF could carry NaN bit patterns
                t_ = hxp.tile([P, NB, ROW], bf16, tag="hx", name="hx")
                nc.vector.memset(t_[:, :, EL + 2 * H:ROW], 0.0)
                return t_

            def stage_a_tile(lx, t, xT_src, rhs_sb, hx_sb):
                h_ps = psA.tile([P, 512], f32, space="PSUM", tag="A",
                                name="h_ps")[:, 0:C + 2 * H]
                sl = slice(t * P, (t + 1) * P)
                nc.tensor.matmul(h_ps[:], xT_src[:, 0, sl], rhs_sb[:, 0, :],
                                 start=True, stop=False)
                nc.tensor.matmul(h_ps[:], xT_src[:, 1, sl], rhs_sb[:, 1, :],
                                 start=False, stop=False)
                nc.tensor.matmul(h_ps[:], teT_sb[:, sl], rhs_sb[:T, 2, :],
                                 start=False, stop=True)
                # h payload as fp8, hi pair (es|ed) on scalar, lo pair on vector
                nc.scalar.activation(hx_sb[:, t, 0:EH].bitcast(f8e4),
                                     h_ps[:, 0:C], ActF.Copy)
                nc.scalar.activation(hx_sb[:, t, EH:EH + 2 * H],
                                     h_ps[:, C:C + 2 * H], ActF.Copy)
                nc.vector.tensor_tensor(out=hx_sb[:, t, EL:EL + 2 * H],
                                        in0=h_ps[:, C:C + 2 * H],
                                        in1=hx_sb[:, t, EH:EH + 2 * H],
                                        op=AluOp.subtract)

            def new_hx_full(lx):
                return drm.tile([TOT_ROWS, ROW], bf16,
                                tag=f"hxf{lx}", name=f"hx_full{lx}")

            def ag_chunk(lx, hx_sb, hx_full, a):
                lo, hi = CHB[a]
                nc.sync.dma_start(
                    ag_in[lo * P:hi * P].rearrange("(t p) f -> p t f", p=P),
                    hx_sb[:, lo:hi, :])
                nc.gpsimd.collective_compute(
                    "AllGather", AluOp.bypass,
                    replica_groups=[list(range(W))],
                    ins=[ag_in[lo * P:hi * P]],
                    outs=[hx_full[CH_BASE[a]:CH_BASE[a] + W * CH_ROWS[a]]])

            # ---------- clear gather buffers once (stale pad lanes must be finite)
            for _ in range(GBUFS):
                g0 = gat.tile([P, ct, ROW], bf16, tag="G")
                nc.vector.memset(g0[:], 0.0)

            pseq = [0]   # prepped-gather sequence number (fire order)

            def gather_prep(b, hx_full):
                """SWDGE descriptor gen only (no data dep on hx_full). Returns
                (G, seq): the paired trigger fires the transfer; consumers
                gate on gsem >= 16*seq. In non-prep mode, a plain gather."""
                G = gat.tile([P, ct, ROW], bf16, tag="G", name="G")
                if use_prep:
                    pseq[0] += 1
                    nc.gpsimd.dma_gather(
                        G[:], hx_full[:], isrc_sb[:, b, :], cap, cap, ROW,
                        single_packet=False, prepare_only=True, sem=gsem)
                    return G, pseq[0]
                nc.gpsimd.dma_gather(
                    G[:], hx_full[:], isrc_sb[:, b, :], cap, cap, ROW,
                    single_packet=False)
                return G, None

            def gather_plain(b, hx_full):
                """Non-prepped gather: its in-engine wait on the AllGather
                writes of hx_full also orders every later trigger_dma on the
                GPSIMD queue after the AllGather. Runs on SWDGE queue 1 so its
                self-triggered ring entry never lands behind untriggered
                prepared entries on queue 0."""
                G = gat.tile([P, ct, ROW], bf16, tag="G", name="G")
                nc.gpsimd.dma_gather(
                    G[:], hx_full[:], isrc_sb[:, b, :], cap, cap, ROW,
                    single_packet=False, queue_num=1 if use_prep else 0)
                return G, None

            def gather_fire():
                if use_prep:
                    nc.gpsimd.trigger_dma(count=None)

            # ---------- x0T = relu(Wi'.T @ xiT), interleaved with stage A l=0
            rhs_sb = sta.tile([P, 3, C + 2 * H], f32r, tag="rhs")
            nc.sync.dma_start(rhs_sb[:], rhs_d[0].rearrange("k p f -> p k f"))
            hx_sb = new_hx()
            xT = xtp.tile([P, 2, NLOC], f32r, tag="xT")
            for j in range(NCH):
                xi_sb = sml.tile([FEAT, 512], f32r, tag="xi")
                nc.sync.dma_start(xi_sb[:], xiT[:, j * 512:(j + 1) * 512])
                for k in range(2):
                    x0_ps = psA.tile([P, 512], f32, space="PSUM", tag="A")
                    nc.tensor.matmul(x0_ps[:], wi_sb[:, k, :], xi_sb[:],
                                     start=True, stop=True)
                    nc.scalar.activation(xT[:, k, j * 512:(j + 1) * 512], x0_ps[:],
                                         ActF.Relu)
                for t in range(4 * j, 4 * j + 4):
                    stage_a_tile(0, t, xT, rhs_sb, hx_sb)
                if j == 1:
                    hx_full = new_hx_full(0)
                    ag_chunk(0, hx_sb, hx_full, 0)
                elif j == 3:
                    ag_chunk(0, hx_sb, hx_full, 1)
            ag_chunk(0, hx_sb, hx_full, 2)
            ag_chunk(0, hx_sb, hx_full, 3)
            if debug:
                nc.sync.dma_start(dbg_hx[:], hx_sb[:])
            # descriptor-gen for blocks 1..PRE runs during the AllGather;
            # block 0 is gathered plain (its AG wait orders the triggers)
            pend = [gather_prep(b, hx_full) for b in range(1, PRE + 1)]

            # ---------------- layers
            for l in range(l_run):
                last = (l + 1 == l_run)
                if not last:
                    rhs_next = sta.tile([P, 3, C + 2 * H], f32r, tag="rhs")
                    nc.sync.dma_start(rhs_next[:],
                                      rhs_d[l + 1].rearrange("k p f -> p k f"))
                    hx_next = new_hx()
                xT_new = xtp.tile([P, 2, NLOC], f32r, tag="xT")

                def consume(b, G, seq=None):
                    """Logits + weighted-feature build for block b (G gathered)."""
                    S_sb = ohp.tile([P, ct, P], f8, tag="S")
                    nc.sync.dma_start(S_sb[:], S_d[b])
                    St_sb = ohp.tile([P, ct, P], f8, tag="St")
                    nc.sync.dma_start(St_sb[:], St_d[b])

                    # ed-expansion: ED[e, 0:8]=ed_hi, [8:16]=junk, [16:24]=ed_lo
                    EDt = psA.tile([P, 512], f32, space="PSUM", tag="A",
                                   name="EDt")[:, 0:ct * 24]
                    EDv = EDt.rearrange("p (t k) -> p t k", k=24)
                    for t in range(ct):
                        nc.tensor.matmul(EDv[:, t, :], St_sb[:, t, :],
                                         hx_sb[:, b, EH + H:EH + 4 * H],
                                         start=True, stop=True)

                    # logits = es_hi + es_lo + ed_hi + ed_lo; leaky; exp.
                    # The first G reader carries the explicit DMA-completion
                    # gate for prepped gathers; later G readers (on DVE) are
                    # ordered behind it.
                    LG = edg.tile([P, ct, H], f32, tag="LG")
                    lg_add = nc.vector.tensor_tensor(
                        out=LG[:], in0=G[:, :, EH:EH + H],
                        in1=G[:, :, EL:EL + H], op=AluOp.add)
                    if seq is not None:
                        lg_add._wait_ge(gsem, 16 * seq)
                    nc.vector.tensor_tensor(out=LG[:], in0=LG[:], in1=EDv[:, :, 0:8],
                                            op=AluOp.add)
                    nc.vector.tensor_tensor(out=LG[:], in0=LG[:], in1=EDv[:, :, 16:24],
                                            op=AluOp.add)
                    LR = edg.tile([P, ct, H], f32, tag="LR")
                    nc.vector.scalar_tensor_tensor(
                        out=LR[:], in0=LG[:], scalar=NEG, in1=LG[:],
                        op0=AluOp.mult, op1=AluOp.max)
                    # WF = [ ex * h  |  ex ]  (h read as fp8 from the G row)
                    WF = edg.tile([P, ct, C + H], bf16, tag="WF")
                    nc.scalar.activation(WF[:, :, C:C + H], LR[:], ActF.Exp)
                    nc.vector.tensor_tensor(
                        out=WF[:, :, 0:C].rearrange("p n (d h) -> p n d h", h=H),
                        in0=G[:, :, 0:EH].bitcast(f8e4)
                              .rearrange("p n (d h) -> p n d h", h=H),
                        in1=WF[:, :, C:C + H].rearrange("p n (o h) -> p n o h", o=1)
                              .broadcast_to([P, ct, D, H]),
                        op=AluOp.mult)
                    return S_sb, WF

                def back(b, S_sb, WF):
                    """Scatter + normalize + node stage (+ next-layer stage A)."""
                    msg_ps = psB.tile([P, C + H], f32, space="PSUM", tag="msg")
                    for t in range(ct):
                        nc.tensor.matmul(msg_ps[:], S_sb[:, t, :], WF[:, t, :],
                                         start=(t == 0), stop=(t == ct - 1))

                    # normalize + relu -> node_new (fused: relu(msg)*recip)
                    recip = sml.tile([P, H], f32, tag="recip")
                    nc.vector.tensor_scalar(out=recip[:], in0=msg_ps[:, C:C + H],
                                            scalar1=1e-12, scalar2=None, op0=AluOp.add)
                    nc.vector.reciprocal(recip[:], recip[:])
                    node_new = nod.tile([P, C], f32, tag="nn")
                    nc.vector.scalar_tensor_tensor(
                        out=node_new[:].rearrange("p (d h) -> p d h", h=H),
                        in0=msg_ps[:, 0:C].rearrange("p (d h) -> p d h", h=H),
                        scalar=0.0,
                        in1=recip[:].rearrange("p (o h) -> p o h", o=1)
                              .broadcast_to([P, D, H]),
                        op0=AluOp.max, op1=AluOp.mult)

                    # nnT via its own transpose group (for the Wqk scores)
                    nt_ps = psB.tile([P, 2, P], f32, space="PSUM", tag="scat",
                                     name="nt_ps", bufs=2)
                    nnT = [sml.tile([P, P], f32r, tag=f"nnT{k}", name=f"nnT{k}")
                           for k in range(2)]
                    for k in range(2):
                        nc.tensor.matmul(nt_ps[:, k, :], node_new[:, k * P:(k + 1) * P],
                                         ident[:].bitcast(f32), is_transpose=True,
                                         start=True, stop=True)
                        nc.scalar.activation(nnT[k][:], nt_ps[:, k, :], ActF.Copy)
                    # scores = node' @ Wqk'  -> [128 nodes, MS]
                    sc_ps = psB.tile([P, MS], f32, space="PSUM", tag="scat", bufs=2)
                    nc.tensor.matmul(sc_ps[:], nnT[0][:], wqk_sb[:, l * 2 + 0, :],
                                     start=True, stop=False)
                    nc.tensor.matmul(sc_ps[:], nnT[1][:], wqk_sb[:, l * 2 + 1, :],
                                     start=False, stop=True)
                    # softmax over MS (no max-sub; scores bounded)
                    attn = sml.tile([P, MS], f32, tag="attn")
                    dnm = sml.tile([P, 1], f32, tag="dnm")
                    nc.scalar.activation(attn[:], sc_ps[:], ActF.Exp, accum_out=dnm[:])
                    nc.vector.reciprocal(dnm[:], dnm[:])
                    nc.vector.tensor_scalar(out=attn[:], in0=attn[:], scalar1=dnm[:, 0:1],
                                            scalar2=None, op0=AluOp.mult)
                    # attnT [MS, 128]
                    at_ps = psB.tile([MS, P], f32, space="PSUM", tag="scat",
                                     name="at_ps", bufs=2)
                    nc.tensor.matmul(at_ps[:], attn[:], ident[:].bitcast(f32),
                                     is_transpose=True, start=True, stop=True)
                    attnT = sml.tile([MS, P], f32r, tag="attnT")
                    nc.scalar.activation(attnT[:], at_ps[:], ActF.Copy)
                    # accumulate attnvT + xT + nnT into outT psum; relu -> xT_new
                    o_all = psC.tile([P, 2, 512], f32, space="PSUM", tag="ops")
                    o_ps = [o_all[:, k, 0:P] for k in range(2)]
                    sl = slice(b * P, (b + 1) * P)
                    for k in range(2):
                        nc.tensor.matmul(o_ps[k], v_sb[:, l, k * P:(k + 1) * P],
                                         attnT[:], start=True, stop=False)
                        nc.tensor.matmul(o_ps[k], ident[:], xT[:, k, sl],
                                         start=False, stop=False)
                        nc.tensor.matmul(o_ps[k], node_new[:, k * P:(k + 1) * P],
                                         ident[:].bitcast(f32), is_transpose=True,
                                         start=False, stop=True)
                        nc.scalar.activation(xT_new[:, k, sl], o_ps[k], ActF.Relu)

                    # software-pipelined stage A for the next layer
                    if not last:
                        stage_a_tile(l + 1, b, xT_new, rhs_next, hx_next)
                        if b == CHB[0][1] - 1:
                            nonlocal_state["hxf_next"] = new_hx_full(l + 1)
                            ag_chunk(l + 1, hx_next, nonlocal_state["hxf_next"], 0)
                        elif b == CHB[1][1] - 1:
                            ag_chunk(l + 1, hx_next, nonlocal_state["hxf_next"], 1)
                        elif b == CHB[2][1] - 1:
                            ag_chunk(l + 1, hx_next, nonlocal_state["hxf_next"], 2)

                # one-block-deep software pipeline: consume(b) || back(b-1),
                # with gather descriptor-gen running PRE blocks ahead
                nonlocal_state = {}
                carry = None
                for b in range(NB):
                    if b == 0:
                        # plain gather: waits the AllGather in-engine, which
                        # also orders every later trigger after the AG
                        G_cur, seq_cur = gather_plain(0, hx_full)
                        gather_fire()            # fires preps for blocks 1..PRE
                    else:
                        G_cur, seq_cur = pend.pop(0)
                    if 1 <= b < NB - PRE:        # preps for blocks PRE+1..19
                        pend.append(gather_prep(b + PRE, hx_full))
                        gather_fire()
                    elif b >= NB - PRE and not last and use_prep:
                        # descriptor-gen for next layer's blocks 1..PRE runs
                        # during this layer's tail; fired only after the next
                        # layer's plain gather has waited out the AllGather
                        nonlocal_state.setdefault("pend_next", []).append(
                            gather_prep(b - (NB - PRE) + 1,
                                        nonlocal_state["hxf_next"]))
                    if debug and l == 0 and b == 1:
                        nc.sync.dma_start(dbg_g[:], G_cur[:])
                    nxt = consume(b, G_cur, seq_cur)
                    if b >= 1:
                        back(b - 1, *carry)
                    carry = nxt
                back(NB - 1, *carry)
                if not last:
                    ag_chunk(l + 1, hx_next, nonlocal_state["hxf_next"], 3)
                    hx_full = nonlocal_state["hxf_next"]
                    if use_prep:
                        pend = nonlocal_state["pend_next"]
                    else:
                        pend = [gather_prep(bb, hx_full)
                                for bb in range(1, PRE + 1)]
                    hx_sb = hx_next
                    rhs_sb = rhs_next
                xT = xT_new
                if debug and l == 0:
                    nc.sync.dma_start(dbg_xt[:], xT[:].bitcast(f32))

            # ---------- pooled partial (exclude per-block pad slots)
            pr1 = sml.tile([P, 2, NB], f32, tag="pr1")
            nc.vector.tensor_reduce(
                out=pr1[:],
                in_=xT[:].rearrange("p k (b q) -> p k b q", q=P)[:, :, :, 0:NREAL],
                axis=AxL.X, op=AluOp.add)
            pooled = sml.tile([P, 2], f32, tag="pooled")
            nc.vector.tensor_reduce(out=pooled[:], in_=pr1[:],
                                    axis=AxL.X, op=AluOp.add)
            nc.sync.dma_start(pooled_d[:], pooled[:])

    nc.compile()
    return nc


# ------------------------------------------------------------------ driver

_CACHED = {}
LAST_EXEC_NS = None


def kernel(**inputs):
    x0 = np.asarray(inputs["x_initial_nodes"], np.float32)
    te = np.asarray(inputs["time_embedding"], np.float32)
    wi = np.asarray(inputs["Wi"], np.float32)[:, PERM].copy()
    bi = np.asarray(inputs["bi"], np.float32)
    assert np.abs(bi).max() == 0.0, "kernel assumes bi == 0"
    rhs, wqk, v = preprocess_weights(inputs)
    cap, per_core, perms = preprocess_graph(np.asarray(inputs["edge_index"]))
    ct = cap // P

    ident = np.eye(P, dtype=np.float32)

    import os
    f8_onehot = os.environ.get("KERNEL_F8", "1") == "1"
    # prepare_only gather pipelining is disabled by default: the deferred
    # trigger path under-synchronizes against the AllGather on HW (NaN via
    # torn fp8 reads). The plain path keeps the chunked-AllGather overlap.
    use_prep = os.environ.get("KERNEL_PREP", "0") == "1"
    if not f8_onehot:
        import ml_dtypes
        per_core = [(isrc, S.astype(ml_dtypes.bfloat16), St.astype(ml_dtypes.bfloat16))
                    for isrc, S, St in per_core]

    in_maps = []
    for c in range(W):
        pm = perms[c]
        sel = np.where(pm >= 0, pm, 0)
        mask = (pm >= 0).astype(np.float32)
        xiT = (x0[c * NSH:(c + 1) * NSH].T[:, sel] * mask).astype(np.float32)
        teT = (te[c * NSH:(c + 1) * NSH].T[:, sel] * mask).astype(np.float32)
        isrc, S, St = per_core[c]
        in_maps.append({
            "xiT": xiT, "teT": teT, "wi": wi,
            "rhs": rhs, "wqk": wqk, "v": v,
            "ident": ident,
            "isrc": isrc, "S": S, "St": St,
        })

    debug = os.environ.get("KERNEL_DEBUG", "0") == "1"
    key = (ct, f8_onehot, use_prep, debug)
    if key not in _CACHED:
        _CACHED[key] = build_nc(ct, f8_onehot=f8_onehot, use_prep=use_prep,
                                debug=debug)
    nc = _CACHED[key]
    trace = os.environ.get("KERNEL_TRACE", "0") == "1"
    tdir = os.environ.get("KERNEL_TRACE_DIR") or None
    res = run_bass_kernel_spmd(nc, in_maps, core_ids=list(range(W)), trace=trace,
                               tmpdir=tdir)
    global LAST_EXEC_NS, LAST_RES
    LAST_EXEC_NS = res.exec_time_ns
    LAST_RES = res

    # feature j_new = k*128 + p
    pooled_new = np.zeros(C, np.float64)
    for c in range(W):
        po = res.results[c]["pooled"].astype(np.float64)
        pooled_new[0:P] += po[:, 0]
        pooled_new[P:C] += po[:, 1]
    pooled_old = np.empty(C, np.float64)
    pooled_old[PERM] = pooled_new
    pooled_old /= N

    mem = np.asarray(inputs["global_memory"], np.float32)
    mem_pooled = mem.mean(axis=0)
    final = np.concatenate([pooled_old.astype(np.float32), mem_pooled])
    out = final @ np.asarray(inputs["Wc"], np.float32) + np.asarray(inputs["bc"], np.float32)
    return out.astype(np.float32)


if __name__ == "__main__":
    import reference
    inp = {k: np.asarray(v) for k, v in reference.setup_inputs().items()}
    got = kernel(**inp)
    exp = np.asarray(reference.reference(**reference.setup_inputs()))
    err = np.abs(got - exp).max() / (np.abs(exp).max() + 1e-12)
    print("kernel:", got, "\nref:   ", exp, "\nrel err:", err)

